# revision 24
# baseline (speedup 1.0000x reference)
"""Trainium2 Bass kernel for y = x @ W^T + b  (B=4096, IN=OUT=2048, fp32).

Sharding: 4-way batch x 2-way out_features across 8 NeuronCores.  Each
core computes a [1024, 1024] output block from x^T [2048, 1024] and
W^T [2048, 1024] shards, both host-packed into ONE dram tensor
kx = [W^T | x^T] ([2048, 2048]) so each k-tile (128 contraction rows)
arrives in a single DMA with a single completion semaphore -- the first
matmul of every k-block then needs exactly one sync wait.

Inputs are cast to bf16 on the host (matmul runs at the same 1 cycle/row
as fp32r but halves HBM traffic; rel err ~2e-3 vs the 2e-2 gate).  The
output is stored bf16 and upcast on the host.

Structure per core:
  - 8 warmup matmuls on uninitialized scratch ramp the PE p-state and
    keep it busy until the first DMA's completion semaphore resolves
    (~11.2us: semaphore increments pipeline globally at ~1.6us/DMA)
  - phase A (batch rows 0-511): k-major over 8 PSUM banks, paced by
    one-DMA-per-k-tile arrivals; epilogue interleaved into the k15
    block so phase B starts stall-free
  - phase B (rows 512-1023): bank-major (all inputs resident), each
    bank's add+store fires as it stops; only the last bank's split
    add+store sits in the tail

Constraint: a Matmult on TRN2 supports only ONE sync wait; the
legalizer below splits multi-waits into EventSemaphore prefixes on the
issuing engine.
"""

import os

import numpy as np

P = 128
B, IN, OUT = 4096, 2048, 2048
MB_SPLIT, NB_SPLIT = 4, 2  # batch-split x out-split = 8 cores
BM = B // MB_SPLIT  # 1024 batch rows per core
NO = OUT // NB_SPLIT  # 1024 out cols per core
KT = IN // P  # 16 k-tiles
MT = BM // P  # 8 m-tiles
NFREE = 512  # PSUM bank free dim (fp32)
NT = NO // NFREE  # 2 n-tiles
N_CORES = 8
PH_M = MT // 2  # 4 m-tiles per phase
KXW = NO + BM  # 2048 packed cols: [W (1024) | x (1024)]

MM_DT = os.environ.get("BASS_MM_DT", "bfloat16")
N_WARMUP = int(os.environ.get("BASS_WARMUP", "8"))

_CACHE = {}


def _np_dt(mm_dt_name):
    if mm_dt_name == "bfloat16":
        import ml_dtypes
        return ml_dtypes.bfloat16
    return np.float32


def _build(mm_dt_name: str):
    import concourse.bass as bass
    import concourse.mybir as mybir
    import concourse.tile as tile

    if mm_dt_name == "fp8hl":
        return _build_fp8()

    mmdt = getattr(mybir.dt, mm_dt_name)
    out_dt = mybir.dt.bfloat16 if mm_dt_name == "bfloat16" else mybir.dt.float32
    f32 = mybir.dt.float32

    nc = bass.Bass("TRN2", target_bir_lowering=False, debug=False,
                   num_devices=N_CORES)
    kx = nc.dram_tensor("kx", [IN, KXW], mmdt, kind="ExternalInput")
    bi = nc.dram_tensor("bi", [NO], f32, kind="ExternalInput")
    y = nc.dram_tensor("y", [BM, NO], out_dt, kind="ExternalOutput")

    kx_r = kx.ap().rearrange("(k p) c -> p k c", p=P)  # [128, 16, 2048]
    y_ap = y.ap()

    # Raw (non-Tile) sbuf scratch for PE warmup, allocated outside the
    # TileContext: the dep tracker adds no producer waits and no release
    # checks.  Contents are garbage; warmup results are discarded.
    dummy_ap = nc.alloc_sbuf_tensor("warm_dummy", [P, NFREE], mmdt).ap()

    XOFF = NO  # x columns start after the 1024 W columns

    with tile.TileContext(nc) as tc:
        with (
            tc.tile_pool(name="sp", bufs=1) as sp,
            tc.tile_pool(name="ps", bufs=1, space="PSUM") as ps,
        ):
            KX = sp.tile([P, KT, KXW], mmdt, tag="kxt", name="KX")
            bias_sb = sp.tile([P, NO], f32, tag="bias", name="bias_sb")

            # DMA-completion semaphores trickle through a GLOBAL ~1.6us-
            # per-DMA pipeline (16 engine increments x ~100ns, serial in
            # completion order), so the early schedule minimizes DMA
            # count: ONE contiguous piece [w0|xa0] gates the first
            # matmul, then one full k-tile per DMA alternating over the
            # two HWDGE queues (data transfers lead the semaphores, and
            # consumption at 1.73us/tile outpaces the 1.6us/DMA sem
            # rate, so the lead grows).  xb0 and bias are only needed
            # from phase A's epilogue on, so they go last.
            nc.sync.dma_start(KX[:, 0, :XOFF + NFREE],
                              kx_r[:, 0, :XOFF + NFREE],
                              single_packet=True)  # w0 + xa0
            for k in range(1, KT):
                eng = nc.scalar if k % 2 == 1 else nc.sync
                eng.dma_start(KX[:, k, :], kx_r[:, k, :])
            nc.scalar.dma_start(KX[:, 0, XOFF + NFREE:],
                                kx_r[:, 0, XOFF + NFREE:])  # xb0
            nc.scalar.dma_start(bias_sb[:],
                                bi.ap()[None, :].to_broadcast((P, NO)))

            psum = {}
            for m in range(PH_M):
                for n in range(NT):
                    psum[(m, n)] = ps.tile([P, NFREE], f32,
                                           tag=f"ps{m}_{n}",
                                           name=f"psum_{m}_{n}")

            # PE p-state warmup while the first DMAs land.  dummy is
            # UNINITIALIZED on purpose: no producer => no waits, so the
            # PE starts the moment its preamble barrier clears.  The
            # results (possibly NaN) land in banks that the first real
            # matmul resets via start=True.
            banks = list(psum.values())
            for i in range(N_WARMUP):
                nc.tensor.matmul(banks[i % len(banks)][:],
                                 lhsT=dummy_ap[:, :P], rhs=dummy_ap[:, :],
                                 start=True, stop=True)

            groups = [(m, n) for m in range(PH_M) for n in range(NT)]

            for phase in range(2):
                xbase = XOFF + phase * (PH_M * P)
                if phase == 0:
                    # k-major: paced by the incoming k-tile DMAs.  k0 is
                    # n-major so the n=1 matmuls give the w0n1 DMA time.
                    # The epilogue is interleaved into the k15 block so
                    # each bank's add fires the moment it stops and
                    # phase B starts with zero boundary stall.
                    for k in range(KT):
                        korder = ([(m, n) for n in range(NT)
                                   for m in range(PH_M)] if k == 0
                                  else groups)
                        for m, n in korder:
                            nc.tensor.matmul(
                                psum[(m, n)][:],
                                lhsT=KX[:, k,
                                        xbase + m * P:xbase + (m + 1) * P],
                                rhs=KX[:, k, n * NFREE:(n + 1) * NFREE],
                                start=(k == 0),
                                stop=(k == KT - 1),
                            )
                            if k == KT - 1:
                                ot = sp.tile([P, NFREE], out_dt,
                                             tag=f"o{phase}_{m}_{n}",
                                             name=f"ota_{phase}_{m}_{n}")
                                nc.vector.tensor_add(
                                    ot[:], psum[(m, n)][:],
                                    bias_sb[:, n * NFREE:(n + 1) * NFREE])
                                row0 = (phase * PH_M + m) * P
                                nc.sync.dma_start(
                                    y_ap[row0:row0 + P,
                                         n * NFREE:(n + 1) * NFREE],
                                    ot[:])
                else:
                    # bank-major: every input is resident, so run each
                    # bank's full k-accumulation consecutively.  Banks
                    # then stop ~3.5us apart and all but the last
                    # add+store overlap the stream instead of the tail.
                    for m, n in groups:
                        for k in range(KT):
                            nc.tensor.matmul(
                                psum[(m, n)][:],
                                lhsT=KX[:, k,
                                        xbase + m * P:xbase + (m + 1) * P],
                                rhs=KX[:, k, n * NFREE:(n + 1) * NFREE],
                                start=(k == 0),
                                stop=(k == KT - 1),
                            )
                        ot = sp.tile([P, NFREE], out_dt,
                                     tag=f"o{phase}_{m}_{n}",
                                     name=f"otb_{phase}_{m}_{n}")
                        row0 = (phase * PH_M + m) * P
                        last = (m, n) == groups[-1]
                        # Last bank: two half adds/stores to shorten the
                        # critical tail chain (half-add -> half-store
                        # overlaps the second half-add).
                        halves = 2 if last else 1
                        hw = NFREE // halves
                        for h in range(halves):
                            c0 = n * NFREE + h * hw
                            nc.vector.tensor_add(
                                ot[:, h * hw:(h + 1) * hw],
                                psum[(m, n)][:, h * hw:(h + 1) * hw],
                                bias_sb[:, c0:c0 + hw])
                            # last half on the OTHER queue: the two
                            # stores' completion-sem pipelines then run
                            # in parallel and teardown starts earlier
                            eng = nc.scalar if h == halves - 1 else nc.sync
                            eng.dma_start(
                                y_ap[row0:row0 + P, c0:c0 + hw],
                                ot[:, h * hw:(h + 1) * hw])

    _strip_redundant_pe_waits(nc)
    _legalize_multi_waits(nc)
    _check_matmul_waits(nc)
    return nc


def _build_fp8():
    """fp8 e4m3 hi/lo 3-term scheme with DoubleRow (0.5 cycles/row).

    x = xh + xl, W = wh + wl (each e4m3; xl/wl quantize the residual).
    y ~= xh@wh + xh@wl + xl@wh  (xl@wl ~0.06%^2, dropped) -- measured
    rel err ~1.3e-3, well under the 2e-2 gate, at 1.5 cycles/row vs
    bf16's 2.0 (3 half-rate passes vs 2... vs 1 full-rate pass): PE
    time 41us vs 54.6us.

    DoubleRow contracts 256 k-values per pass: lhsT [128, 2, M],
    rhs [128, 2, N], out += sum_i lhsT[:,i].T @ rhs[:,i].  The dim-1
    pair indexes two adjacent k-tiles, realized as strided 3-D APs into
    one big SBUF tile -- no data interleaving needed.
    """
    import concourse.bass as bass
    import concourse.mybir as mybir
    import concourse.tile as tile

    f8 = mybir.dt.float8e4
    f32 = mybir.dt.float32
    bf16 = mybir.dt.bfloat16
    DR = mybir.MatmulPerfMode.DoubleRow
    KXW8 = 2 * NO + 2 * BM  # [Wh | Wl | Xh | Xl]
    WH, WL, XH, XL = 0, NO, 2 * NO, 2 * NO + BM
    DT_ = KT // 2  # 8 double-k tiles

    nc = bass.Bass("TRN2", target_bir_lowering=False, debug=False,
                   num_devices=N_CORES)
    kx = nc.dram_tensor("kx", [IN, KXW8], f8, kind="ExternalInput")
    bi = nc.dram_tensor("bi", [NO], f32, kind="ExternalInput")
    y = nc.dram_tensor("y", [BM, NO], bf16, kind="ExternalOutput")

    kx_r = kx.ap().rearrange("(k p) c -> p k c", p=P)  # [128, 16, 4096]
    y_ap = y.ap()

    # Raw (non-Tile) warmup scratch; see _build.
    dummy_ap = nc.alloc_sbuf_tensor(
        "warm_dummy", [P, 2 * NFREE], f8).ap().rearrange(
        "p (two f) -> p two f", two=2)

    with tile.TileContext(nc) as tc:
        with (
            tc.tile_pool(name="sp", bufs=1) as sp,
            tc.tile_pool(name="ps", bufs=1, space="PSUM") as ps,
        ):
            KX = sp.tile([P, KT, KXW8], f8, tag="kxt", name="KX")
            bias_sb = sp.tile([P, NO], f32, tag="bias", name="bias_sb")

            # d0 (k-tiles 0+1) lands in 4 pieces ordered by first use,
            # split across both HWDGE engines so the two pieces gating
            # the first matmul (Xh on Sync, Wh on Act) resolve their
            # completion-semaphore pipelines in parallel; d>=1 is one
            # DMA per k-pair => one wait per d-block.
            nc.sync.dma_start(KX[:, 0:2, XH:XH + BM],
                              kx_r[:, 0:2, XH:XH + BM])
            nc.scalar.dma_start(KX[:, 0:2, WH:WH + NO],
                                kx_r[:, 0:2, WH:WH + NO])
            nc.scalar.dma_start(KX[:, 0:2, WL:WL + NO],
                                kx_r[:, 0:2, WL:WL + NO])
            nc.scalar.dma_start(KX[:, 0:2, XL:XL + BM],
                                kx_r[:, 0:2, XL:XL + BM])
            for d in range(1, DT_):
                eng = nc.sync if d % 2 == 1 else nc.scalar
                eng.dma_start(KX[:, 2 * d:2 * d + 2, :],
                              kx_r[:, 2 * d:2 * d + 2, :])
            nc.scalar.dma_start(bias_sb[:],
                                bi.ap()[None, :].to_broadcast((P, NO)))

            psum = {}
            for m in range(PH_M):
                for n in range(NT):
                    psum[(m, n)] = ps.tile([P, NFREE], f32,
                                           tag=f"ps{m}_{n}",
                                           name=f"psum_{m}_{n}")

            # Uninitialized-dummy warmup: no producer => no waits; PE
            # ramps from the moment its preamble barrier clears.
            banks = list(psum.values())
            for i in range(2 * N_WARMUP):
                nc.tensor.matmul(banks[i % len(banks)][:],
                                 lhsT=dummy_ap[:, :, :P], rhs=dummy_ap[:, :, :],
                                 start=True, stop=True, perf_mode=DR)

            # terms: (x col offset, w col offset, term id)
            T0, T1, T2 = (XH, WH, 0), (XH, WL, 1), (XL, WH, 2)

            def mm(d, t, m, n, xb):
                xo, wo, tid = t
                ks = slice(2 * d, 2 * d + 2)
                nc.tensor.matmul(
                    psum[(m, n)][:],
                    lhsT=KX[:, ks, xo + xb + m * P:xo + xb + (m + 1) * P],
                    rhs=KX[:, ks, wo + n * NFREE:wo + (n + 1) * NFREE],
                    start=(d == 0 and tid == 0),
                    stop=(d == DT_ - 1 and tid == 2),
                    perf_mode=DR,
                )

            def epilogue(phase, m, n):
                ot = sp.tile([P, NFREE], bf16, tag=f"o{phase}_{m}_{n}",
                             name=f"ot_{phase}_{m}_{n}")
                nc.vector.tensor_add(
                    ot[:], psum[(m, n)][:],
                    bias_sb[:, n * NFREE:(n + 1) * NFREE])
                row0 = (phase * PH_M + m) * P
                nc.sync.dma_start(
                    y_ap[row0:row0 + P, n * NFREE:(n + 1) * NFREE], ot[:])

            # Phase A: d-major, paced by incoming k-pair DMAs.  d0 is
            # term-major (Wl/Xl pieces may still be in flight); d>=1
            # m-major so T0/T1 share the Xh weight load.
            for d in range(DT_):
                if d == 0:
                    order = [(t, m, n) for t in (T0, T1, T2)
                             for m in range(PH_M) for n in range(NT)]
                else:
                    order = [(t, m, n) for m in range(PH_M)
                             for t in (T0, T1, T2) for n in range(NT)]
                for t, m, n in order:
                    mm(d, t, m, n, 0)
            for m in range(PH_M):
                for n in range(NT):
                    epilogue(0, m, n)

            # Phase B: bank-major (everything resident): each bank runs
            # its full 24-matmul accumulation consecutively, so banks
            # stop staggered and only the last add+store is in the tail.
            xb = PH_M * P
            for m in range(PH_M):
                for n in range(NT):
                    for d in range(DT_):
                        for t in (T0, T1, T2):
                            mm(d, t, m, n, xb)
                    epilogue(1, m, n)

    _strip_redundant_pe_waits(nc)
    _legalize_multi_waits(nc)
    _check_matmul_waits(nc)
    return nc


def _legalize_multi_waits(nc):
    """Split multi-wait instructions into single-wait EventSemaphore
    prefixes on the same engine.

    This walrus pipeline (bass pass list, no lower_sync) supports exactly
    ONE sync wait per instruction.  A chain of EventSemaphore waits on the
    issuing engine followed by the instruction with the final wait is
    semantically identical: the engine's sequencer blocks on each in
    order.
    """
    import copy

    import concourse.mybir as mybir

    m = nc.m
    new_module = copy.replace(m, functions=[])
    counter = [0]
    for function in m.functions:
        new_function = copy.replace(function, blocks=[])
        new_function.set_allocations_from_list(function.allocations)
        for block in function.blocks:
            new_insts = []
            for inst in block.instructions:
                s = inst.sync_info
                if s and s.on_wait and len(s.on_wait) > 1:
                    for w in s.on_wait[:-1]:
                        counter[0] += 1
                        ev = mybir.InstEventSemaphore(
                            name=f"legalize_wait_{counter[0]}",
                            ins=[], outs=[],
                            sync_info=mybir.SyncInfo(on_wait=[w],
                                                     on_update=[]),
                            engine=inst.engine,
                        )
                        new_insts.append(ev)
                    inst.sync_info = mybir.SyncInfo(
                        on_wait=[s.on_wait[-1]], on_update=s.on_update)
                new_insts.append(inst)
            new_function.blocks.append(
                copy.replace(block, instructions=new_insts))
        new_module.functions.append(new_function)
    nc.m = new_module


def _strip_redundant_pe_waits(nc):
    """Drop PE self-waits on matmuls that also wait on the DVE release.

    TRN2 matmuls support one sync wait.  Tile's wait emission is not
    transitively minimal: a PSUM-bank reuse emits both the bank's last PE
    writer (self-engine, redundant: the DVE add that releases the bank
    already waits on that writer) and the DVE release.  Keeping the DVE
    wait preserves the hazard ordering.
    """
    import concourse.mybir as mybir

    for bb in nc.m.functions[0].blocks:
        for inst in bb.instructions:
            if type(inst).__name__ != "InstMatmult":
                continue
            s = inst.sync_info
            if not (s and s.on_wait and len(s.on_wait) > 1):
                continue
            keep = [w for w in s.on_wait if not w.ant_name.startswith("PE")]
            dve = [w for w in keep if w.ant_name.startswith("DVE")]
            if len(keep) == len(s.on_wait) - 1 and dve:
                inst.sync_info = mybir.SyncInfo(on_wait=keep,
                                                on_update=s.on_update)


def _check_matmul_waits(nc):
    """TRN2 compute instructions (Matmult, TensorTensor, ...) support one
    sync wait; walrus codegen hard-fails on more."""
    limited = {"InstMatmult", "InstTensorTensor", "InstTensorScalarPtr",
               "InstActivation", "InstTensorCopy", "InstCopy"}
    bad = []
    for bb in nc.m.functions[0].blocks:
        for inst in bb.instructions:
            if type(inst).__name__ in limited:
                s = inst.sync_info
                nw = len(s.on_wait) if s and s.on_wait else 0
                if nw > 1:
                    bad.append((inst.name, type(inst).__name__,
                                [(w.ant_name, w.wait_value)
                                 for w in s.on_wait]))
    if bad:
        raise RuntimeError(f"{len(bad)} insts with >1 wait: {bad[:8]}")


def make_in_maps(x, weights, bias):
    x = np.asarray(x, dtype=np.float32)
    weights = np.asarray(weights, dtype=np.float32)
    bias = np.asarray(bias, dtype=np.float32)

    xT = np.ascontiguousarray(x.T)  # [IN, B]
    wT = np.ascontiguousarray(weights.T)  # [IN, OUT]

    if MM_DT == "fp8hl":
        import ml_dtypes
        f8 = ml_dtypes.float8_e4m3
        xh = xT.astype(f8)
        xl = (xT - xh.astype(np.float32)).astype(f8)
        wh = wT.astype(f8)
        wl = (wT - wh.astype(np.float32)).astype(f8)
        in_maps = []
        for c in range(N_CORES):
            mb, nb = divmod(c, NB_SPLIT)
            ns = slice(nb * NO, (nb + 1) * NO)
            ms = slice(mb * BM, (mb + 1) * BM)
            kx = np.concatenate(
                [wh[:, ns], wl[:, ns], xh[:, ms], xl[:, ms]], axis=1)
            in_maps.append({
                "kx": np.ascontiguousarray(kx),
                "bi": np.ascontiguousarray(bias[nb * NO:(nb + 1) * NO]),
            })
        return in_maps

    np_dt = _np_dt(MM_DT)
    in_maps = []
    for c in range(N_CORES):
        mb, nb = divmod(c, NB_SPLIT)
        kx = np.concatenate(
            [wT[:, nb * NO:(nb + 1) * NO], xT[:, mb * BM:(mb + 1) * BM]],
            axis=1).astype(np_dt)
        in_maps.append({
            "kx": np.ascontiguousarray(kx),
            "bi": np.ascontiguousarray(bias[nb * NO:(nb + 1) * NO]),
        })
    return in_maps


def gather_output(res):
    out = np.empty((B, OUT), dtype=np.float32)
    for c in range(N_CORES):
        mb, nb = divmod(c, NB_SPLIT)
        out[mb * BM:(mb + 1) * BM,
            nb * NO:(nb + 1) * NO] = np.asarray(
                res.results[c]["y"]).astype(np.float32)
    return out


def kernel(x, weights, bias):
    from concourse.bass_utils import run_bass_kernel_spmd

    if MM_DT not in _CACHE:
        _CACHE[MM_DT] = _build(MM_DT)
    nc = _CACHE[MM_DT]

    in_maps = make_in_maps(x, weights, bias)
    res = run_bass_kernel_spmd(nc, in_maps, core_ids=list(range(N_CORES)))
    return gather_output(res)


# revision 25
# speedup vs baseline: 1.0431x; 1.0431x over previous
"""Trainium2 Bass kernel for y = x @ W^T + b  (B=4096, IN=OUT=2048, fp32).

Sharding: 4-way batch x 2-way out_features across 8 NeuronCores.  Each
core computes a [1024, 1024] output block from x^T [2048, 1024] and
W^T [2048, 1024] shards, both host-packed into ONE dram tensor
kx = [W^T | x^T] ([2048, 2048]) so each k-tile (128 contraction rows)
arrives in a single DMA with a single completion semaphore -- the first
matmul of every k-block then needs exactly one sync wait.

Inputs are cast to bf16 on the host (matmul runs at the same 1 cycle/row
as fp32r but halves HBM traffic; rel err ~2e-3 vs the 2e-2 gate).  The
output is stored bf16 and upcast on the host.

Structure per core:
  - 8 warmup matmuls on uninitialized scratch ramp the PE p-state and
    keep it busy until the first DMA's completion semaphore resolves
    (~11.2us: semaphore increments pipeline globally at ~1.6us/DMA)
  - phase A (batch rows 0-511): k-major over 8 PSUM banks, paced by
    one-DMA-per-k-tile arrivals; epilogue interleaved into the k15
    block so phase B starts stall-free
  - phase B (rows 512-1023): bank-major (all inputs resident), each
    bank's add+store fires as it stops; only the last bank's split
    add+store sits in the tail

Constraint: a Matmult on TRN2 supports only ONE sync wait; the
legalizer below splits multi-waits into EventSemaphore prefixes on the
issuing engine.
"""

import os

import numpy as np

P = 128
B, IN, OUT = 4096, 2048, 2048
MB_SPLIT, NB_SPLIT = 4, 2  # batch-split x out-split = 8 cores
BM = B // MB_SPLIT  # 1024 batch rows per core
NO = OUT // NB_SPLIT  # 1024 out cols per core
KT = IN // P  # 16 k-tiles
MT = BM // P  # 8 m-tiles
NFREE = 512  # PSUM bank free dim (fp32)
NT = NO // NFREE  # 2 n-tiles
N_CORES = 8
PH_M = MT // 2  # 4 m-tiles per phase
KXW = NO + BM  # 2048 packed cols: [W (1024) | x (1024)]

MM_DT = os.environ.get("BASS_MM_DT", "bfloat16")
N_WARMUP = int(os.environ.get("BASS_WARMUP", "8"))

_CACHE = {}


def _np_dt(mm_dt_name):
    if mm_dt_name == "bfloat16":
        import ml_dtypes
        return ml_dtypes.bfloat16
    return np.float32


def _build(mm_dt_name: str):
    import concourse.bass as bass
    import concourse.mybir as mybir
    import concourse.tile as tile

    if mm_dt_name == "fp8hl":
        return _build_fp8()

    mmdt = getattr(mybir.dt, mm_dt_name)
    out_dt = mybir.dt.bfloat16 if mm_dt_name == "bfloat16" else mybir.dt.float32
    f32 = mybir.dt.float32

    nc = bass.Bass("TRN2", target_bir_lowering=False, debug=False,
                   num_devices=N_CORES)
    kx = nc.dram_tensor("kx", [IN, KXW], mmdt, kind="ExternalInput")
    bi = nc.dram_tensor("bi", [NO], f32, kind="ExternalInput")
    y = nc.dram_tensor("y", [BM, NO], out_dt, kind="ExternalOutput")

    kx_r = kx.ap().rearrange("(k p) c -> p k c", p=P)  # [128, 16, 2048]
    y_ap = y.ap()

    # Raw (non-Tile) sbuf scratch for PE warmup, allocated outside the
    # TileContext: the dep tracker adds no producer waits and no release
    # checks.  Contents are garbage; warmup results are discarded.
    dummy_ap = nc.alloc_sbuf_tensor("warm_dummy", [P, NFREE], mmdt).ap()

    XOFF = NO  # x columns start after the 1024 W columns

    with tile.TileContext(nc) as tc:
        with (
            tc.tile_pool(name="sp", bufs=1) as sp,
            tc.tile_pool(name="ps", bufs=1, space="PSUM") as ps,
        ):
            KX = sp.tile([P, KT, KXW], mmdt, tag="kxt", name="KX")
            bias_sb = sp.tile([P, NO], f32, tag="bias", name="bias_sb")

            # DMA-completion semaphores trickle through a GLOBAL ~1.6us-
            # per-DMA pipeline (16 engine increments x ~100ns, serial in
            # completion order), so the early schedule minimizes DMA
            # count: ONE contiguous piece [w0|xa0] gates the first
            # matmul, then one full k-tile per DMA alternating over the
            # two HWDGE queues (data transfers lead the semaphores, and
            # consumption at 1.73us/tile outpaces the 1.6us/DMA sem
            # rate, so the lead grows).  xb0 and bias are only needed
            # from phase A's epilogue on, so they go last.
            nc.sync.dma_start(KX[:, 0, :XOFF + NFREE],
                              kx_r[:, 0, :XOFF + NFREE])  # w0 + xa0
            for k in range(1, KT):
                eng = nc.scalar if k % 2 == 1 else nc.sync
                eng.dma_start(KX[:, k, :], kx_r[:, k, :])
            nc.scalar.dma_start(KX[:, 0, XOFF + NFREE:],
                                kx_r[:, 0, XOFF + NFREE:])  # xb0
            nc.scalar.dma_start(bias_sb[:],
                                bi.ap()[None, :].to_broadcast((P, NO)))

            psum = {}
            for m in range(PH_M):
                for n in range(NT):
                    psum[(m, n)] = ps.tile([P, NFREE], f32,
                                           tag=f"ps{m}_{n}",
                                           name=f"psum_{m}_{n}")

            # PE p-state warmup while the first DMAs land.  dummy is
            # UNINITIALIZED on purpose: no producer => no waits, so the
            # PE starts the moment its preamble barrier clears.  The
            # results (possibly NaN) land in banks that the first real
            # matmul resets via start=True.
            banks = list(psum.values())
            for i in range(N_WARMUP):
                nc.tensor.matmul(banks[i % len(banks)][:],
                                 lhsT=dummy_ap[:, :P], rhs=dummy_ap[:, :],
                                 start=True, stop=True)

            groups = [(m, n) for m in range(PH_M) for n in range(NT)]

            for phase in range(2):
                xbase = XOFF + phase * (PH_M * P)
                if phase == 0:
                    # k-major: paced by the incoming k-tile DMAs.  k0 is
                    # n-major so the n=1 matmuls give the w0n1 DMA time.
                    # The epilogue is interleaved into the k15 block so
                    # each bank's add fires the moment it stops and
                    # phase B starts with zero boundary stall.
                    for k in range(KT):
                        korder = ([(m, n) for n in range(NT)
                                   for m in range(PH_M)] if k == 0
                                  else groups)
                        for m, n in korder:
                            nc.tensor.matmul(
                                psum[(m, n)][:],
                                lhsT=KX[:, k,
                                        xbase + m * P:xbase + (m + 1) * P],
                                rhs=KX[:, k, n * NFREE:(n + 1) * NFREE],
                                start=(k == 0),
                                stop=(k == KT - 1),
                            )
                            if k == KT - 1:
                                ot = sp.tile([P, NFREE], out_dt,
                                             tag=f"o{phase}_{m}_{n}",
                                             name=f"ota_{phase}_{m}_{n}")
                                nc.vector.tensor_add(
                                    ot[:], psum[(m, n)][:],
                                    bias_sb[:, n * NFREE:(n + 1) * NFREE])
                                row0 = (phase * PH_M + m) * P
                                nc.sync.dma_start(
                                    y_ap[row0:row0 + P,
                                         n * NFREE:(n + 1) * NFREE],
                                    ot[:])
                else:
                    # bank-major: every input is resident, so run each
                    # bank's full k-accumulation consecutively.  Banks
                    # then stop ~3.5us apart and all but the last
                    # add+store overlap the stream instead of the tail.
                    for m, n in groups:
                        for k in range(KT):
                            nc.tensor.matmul(
                                psum[(m, n)][:],
                                lhsT=KX[:, k,
                                        xbase + m * P:xbase + (m + 1) * P],
                                rhs=KX[:, k, n * NFREE:(n + 1) * NFREE],
                                start=(k == 0),
                                stop=(k == KT - 1),
                            )
                        ot = sp.tile([P, NFREE], out_dt,
                                     tag=f"o{phase}_{m}_{n}",
                                     name=f"otb_{phase}_{m}_{n}")
                        row0 = (phase * PH_M + m) * P
                        last = (m, n) == groups[-1]
                        # Last bank: asymmetric [384|128] split -- the
                        # big add+store overlaps the tiny second chunk,
                        # so the critical tail chain after the final
                        # matmul is one small add + one small store.
                        # The two stores go on different queues so their
                        # completion-sem pipelines resolve in parallel.
                        pieces = [(0, NFREE - P), (NFREE - P, P)] if last \
                            else [(0, NFREE)]
                        for pi, (off, w_) in enumerate(pieces):
                            c0 = n * NFREE + off
                            nc.vector.tensor_add(
                                ot[:, off:off + w_],
                                psum[(m, n)][:, off:off + w_],
                                bias_sb[:, c0:c0 + w_])
                            eng = nc.scalar if pi == len(pieces) - 1 \
                                else nc.sync
                            eng.dma_start(
                                y_ap[row0:row0 + P, c0:c0 + w_],
                                ot[:, off:off + w_])

    _strip_redundant_pe_waits(nc)
    _legalize_multi_waits(nc)
    _check_matmul_waits(nc)
    return nc


def _build_fp8():
    """fp8 e4m3 hi/lo 3-term scheme with DoubleRow (0.5 cycles/row).

    x = xh + xl, W = wh + wl (each e4m3; xl/wl quantize the residual).
    y ~= xh@wh + xh@wl + xl@wh  (xl@wl ~0.06%^2, dropped) -- measured
    rel err ~1.3e-3, well under the 2e-2 gate, at 1.5 cycles/row vs
    bf16's 2.0 (3 half-rate passes vs 2... vs 1 full-rate pass): PE
    time 41us vs 54.6us.

    DoubleRow contracts 256 k-values per pass: lhsT [128, 2, M],
    rhs [128, 2, N], out += sum_i lhsT[:,i].T @ rhs[:,i].  The dim-1
    pair indexes two adjacent k-tiles, realized as strided 3-D APs into
    one big SBUF tile -- no data interleaving needed.
    """
    import concourse.bass as bass
    import concourse.mybir as mybir
    import concourse.tile as tile

    f8 = mybir.dt.float8e4
    f32 = mybir.dt.float32
    bf16 = mybir.dt.bfloat16
    DR = mybir.MatmulPerfMode.DoubleRow
    KXW8 = 2 * NO + 2 * BM  # [Wh | Wl | Xh | Xl]
    WH, WL, XH, XL = 0, NO, 2 * NO, 2 * NO + BM
    DT_ = KT // 2  # 8 double-k tiles

    nc = bass.Bass("TRN2", target_bir_lowering=False, debug=False,
                   num_devices=N_CORES)
    kx = nc.dram_tensor("kx", [IN, KXW8], f8, kind="ExternalInput")
    bi = nc.dram_tensor("bi", [NO], f32, kind="ExternalInput")
    y = nc.dram_tensor("y", [BM, NO], bf16, kind="ExternalOutput")

    kx_r = kx.ap().rearrange("(k p) c -> p k c", p=P)  # [128, 16, 4096]
    y_ap = y.ap()

    # Raw (non-Tile) warmup scratch; see _build.
    dummy_ap = nc.alloc_sbuf_tensor(
        "warm_dummy", [P, 2 * NFREE], f8).ap().rearrange(
        "p (two f) -> p two f", two=2)

    with tile.TileContext(nc) as tc:
        with (
            tc.tile_pool(name="sp", bufs=1) as sp,
            tc.tile_pool(name="ps", bufs=1, space="PSUM") as ps,
        ):
            KX = sp.tile([P, KT, KXW8], f8, tag="kxt", name="KX")
            bias_sb = sp.tile([P, NO], f32, tag="bias", name="bias_sb")

            # d0 (k-tiles 0+1) lands in 4 pieces ordered by first use,
            # split across both HWDGE engines so the two pieces gating
            # the first matmul (Xh on Sync, Wh on Act) resolve their
            # completion-semaphore pipelines in parallel; d>=1 is one
            # DMA per k-pair => one wait per d-block.
            nc.sync.dma_start(KX[:, 0:2, XH:XH + BM],
                              kx_r[:, 0:2, XH:XH + BM])
            nc.scalar.dma_start(KX[:, 0:2, WH:WH + NO],
                                kx_r[:, 0:2, WH:WH + NO])
            nc.scalar.dma_start(KX[:, 0:2, WL:WL + NO],
                                kx_r[:, 0:2, WL:WL + NO])
            nc.scalar.dma_start(KX[:, 0:2, XL:XL + BM],
                                kx_r[:, 0:2, XL:XL + BM])
            for d in range(1, DT_):
                eng = nc.sync if d % 2 == 1 else nc.scalar
                eng.dma_start(KX[:, 2 * d:2 * d + 2, :],
                              kx_r[:, 2 * d:2 * d + 2, :])
            nc.scalar.dma_start(bias_sb[:],
                                bi.ap()[None, :].to_broadcast((P, NO)))

            psum = {}
            for m in range(PH_M):
                for n in range(NT):
                    psum[(m, n)] = ps.tile([P, NFREE], f32,
                                           tag=f"ps{m}_{n}",
                                           name=f"psum_{m}_{n}")

            # Uninitialized-dummy warmup: no producer => no waits; PE
            # ramps from the moment its preamble barrier clears.
            banks = list(psum.values())
            for i in range(2 * N_WARMUP):
                nc.tensor.matmul(banks[i % len(banks)][:],
                                 lhsT=dummy_ap[:, :, :P], rhs=dummy_ap[:, :, :],
                                 start=True, stop=True, perf_mode=DR)

            # terms: (x col offset, w col offset, term id)
            T0, T1, T2 = (XH, WH, 0), (XH, WL, 1), (XL, WH, 2)

            def mm(d, t, m, n, xb):
                xo, wo, tid = t
                ks = slice(2 * d, 2 * d + 2)
                nc.tensor.matmul(
                    psum[(m, n)][:],
                    lhsT=KX[:, ks, xo + xb + m * P:xo + xb + (m + 1) * P],
                    rhs=KX[:, ks, wo + n * NFREE:wo + (n + 1) * NFREE],
                    start=(d == 0 and tid == 0),
                    stop=(d == DT_ - 1 and tid == 2),
                    perf_mode=DR,
                )

            def epilogue(phase, m, n):
                ot = sp.tile([P, NFREE], bf16, tag=f"o{phase}_{m}_{n}",
                             name=f"ot_{phase}_{m}_{n}")
                nc.vector.tensor_add(
                    ot[:], psum[(m, n)][:],
                    bias_sb[:, n * NFREE:(n + 1) * NFREE])
                row0 = (phase * PH_M + m) * P
                nc.sync.dma_start(
                    y_ap[row0:row0 + P, n * NFREE:(n + 1) * NFREE], ot[:])

            # Phase A: d-major, paced by incoming k-pair DMAs.  d0 is
            # term-major (Wl/Xl pieces may still be in flight); d>=1
            # m-major so T0/T1 share the Xh weight load.
            for d in range(DT_):
                if d == 0:
                    order = [(t, m, n) for t in (T0, T1, T2)
                             for m in range(PH_M) for n in range(NT)]
                else:
                    order = [(t, m, n) for m in range(PH_M)
                             for t in (T0, T1, T2) for n in range(NT)]
                for t, m, n in order:
                    mm(d, t, m, n, 0)
            for m in range(PH_M):
                for n in range(NT):
                    epilogue(0, m, n)

            # Phase B: bank-major (everything resident): each bank runs
            # its full 24-matmul accumulation consecutively, so banks
            # stop staggered and only the last add+store is in the tail.
            xb = PH_M * P
            for m in range(PH_M):
                for n in range(NT):
                    for d in range(DT_):
                        for t in (T0, T1, T2):
                            mm(d, t, m, n, xb)
                    epilogue(1, m, n)

    _strip_redundant_pe_waits(nc)
    _legalize_multi_waits(nc)
    _check_matmul_waits(nc)
    return nc


def _legalize_multi_waits(nc):
    """Split multi-wait instructions into single-wait EventSemaphore
    prefixes on the same engine.

    This walrus pipeline (bass pass list, no lower_sync) supports exactly
    ONE sync wait per instruction.  A chain of EventSemaphore waits on the
    issuing engine followed by the instruction with the final wait is
    semantically identical: the engine's sequencer blocks on each in
    order.
    """
    import copy

    import concourse.mybir as mybir

    m = nc.m
    new_module = copy.replace(m, functions=[])
    counter = [0]
    for function in m.functions:
        new_function = copy.replace(function, blocks=[])
        new_function.set_allocations_from_list(function.allocations)
        for block in function.blocks:
            new_insts = []
            for inst in block.instructions:
                s = inst.sync_info
                if s and s.on_wait and len(s.on_wait) > 1:
                    for w in s.on_wait[:-1]:
                        counter[0] += 1
                        ev = mybir.InstEventSemaphore(
                            name=f"legalize_wait_{counter[0]}",
                            ins=[], outs=[],
                            sync_info=mybir.SyncInfo(on_wait=[w],
                                                     on_update=[]),
                            engine=inst.engine,
                        )
                        new_insts.append(ev)
                    inst.sync_info = mybir.SyncInfo(
                        on_wait=[s.on_wait[-1]], on_update=s.on_update)
                new_insts.append(inst)
            new_function.blocks.append(
                copy.replace(block, instructions=new_insts))
        new_module.functions.append(new_function)
    nc.m = new_module


def _strip_redundant_pe_waits(nc):
    """Drop PE self-waits on matmuls that also wait on the DVE release.

    TRN2 matmuls support one sync wait.  Tile's wait emission is not
    transitively minimal: a PSUM-bank reuse emits both the bank's last PE
    writer (self-engine, redundant: the DVE add that releases the bank
    already waits on that writer) and the DVE release.  Keeping the DVE
    wait preserves the hazard ordering.
    """
    import concourse.mybir as mybir

    for bb in nc.m.functions[0].blocks:
        for inst in bb.instructions:
            if type(inst).__name__ != "InstMatmult":
                continue
            s = inst.sync_info
            if not (s and s.on_wait and len(s.on_wait) > 1):
                continue
            keep = [w for w in s.on_wait if not w.ant_name.startswith("PE")]
            dve = [w for w in keep if w.ant_name.startswith("DVE")]
            if len(keep) == len(s.on_wait) - 1 and dve:
                inst.sync_info = mybir.SyncInfo(on_wait=keep,
                                                on_update=s.on_update)


def _check_matmul_waits(nc):
    """TRN2 compute instructions (Matmult, TensorTensor, ...) support one
    sync wait; walrus codegen hard-fails on more."""
    limited = {"InstMatmult", "InstTensorTensor", "InstTensorScalarPtr",
               "InstActivation", "InstTensorCopy", "InstCopy"}
    bad = []
    for bb in nc.m.functions[0].blocks:
        for inst in bb.instructions:
            if type(inst).__name__ in limited:
                s = inst.sync_info
                nw = len(s.on_wait) if s and s.on_wait else 0
                if nw > 1:
                    bad.append((inst.name, type(inst).__name__,
                                [(w.ant_name, w.wait_value)
                                 for w in s.on_wait]))
    if bad:
        raise RuntimeError(f"{len(bad)} insts with >1 wait: {bad[:8]}")


def make_in_maps(x, weights, bias):
    x = np.asarray(x, dtype=np.float32)
    weights = np.asarray(weights, dtype=np.float32)
    bias = np.asarray(bias, dtype=np.float32)

    xT = np.ascontiguousarray(x.T)  # [IN, B]
    wT = np.ascontiguousarray(weights.T)  # [IN, OUT]

    if MM_DT == "fp8hl":
        import ml_dtypes
        f8 = ml_dtypes.float8_e4m3
        xh = xT.astype(f8)
        xl = (xT - xh.astype(np.float32)).astype(f8)
        wh = wT.astype(f8)
        wl = (wT - wh.astype(np.float32)).astype(f8)
        in_maps = []
        for c in range(N_CORES):
            mb, nb = divmod(c, NB_SPLIT)
            ns = slice(nb * NO, (nb + 1) * NO)
            ms = slice(mb * BM, (mb + 1) * BM)
            kx = np.concatenate(
                [wh[:, ns], wl[:, ns], xh[:, ms], xl[:, ms]], axis=1)
            in_maps.append({
                "kx": np.ascontiguousarray(kx),
                "bi": np.ascontiguousarray(bias[nb * NO:(nb + 1) * NO]),
            })
        return in_maps

    np_dt = _np_dt(MM_DT)
    in_maps = []
    for c in range(N_CORES):
        mb, nb = divmod(c, NB_SPLIT)
        kx = np.concatenate(
            [wT[:, nb * NO:(nb + 1) * NO], xT[:, mb * BM:(mb + 1) * BM]],
            axis=1).astype(np_dt)
        in_maps.append({
            "kx": np.ascontiguousarray(kx),
            "bi": np.ascontiguousarray(bias[nb * NO:(nb + 1) * NO]),
        })
    return in_maps


def gather_output(res):
    out = np.empty((B, OUT), dtype=np.float32)
    for c in range(N_CORES):
        mb, nb = divmod(c, NB_SPLIT)
        out[mb * BM:(mb + 1) * BM,
            nb * NO:(nb + 1) * NO] = np.asarray(
                res.results[c]["y"]).astype(np.float32)
    return out


def kernel(x, weights, bias):
    from concourse.bass_utils import run_bass_kernel_spmd

    if MM_DT not in _CACHE:
        _CACHE[MM_DT] = _build(MM_DT)
    nc = _CACHE[MM_DT]

    in_maps = make_in_maps(x, weights, bias)
    res = run_bass_kernel_spmd(nc, in_maps, core_ids=list(range(N_CORES)))
    return gather_output(res)
